# revision 4
# baseline (speedup 1.0000x reference)
"""Trainium2 Bass kernel for nn_ExemplarNoAttention (retrieval_knn).

logits[b,c] = log(eps + sum_{e: label[e]==c} exp(-beta * ||x_b - E_e||^2))

Sharding: data-parallel over the batch. Each of the 8 NeuronCores computes
its own 128 queries against the full exemplar bank (replicated, class-sorted
on the host); the host concatenates the per-core (128, 10) outputs. No
collectives: each core's pipeline is fully independent.

Device pipeline per core (one batch tile of 128 queries):
  TensorE : psum[b,e] = 2*beta*<x_b,E_e> - beta*e2_e  (bf16 GEMM, K=65:
            rows 0..63 = features, row 64 = 1 -> -beta*e2 augmentation row;
            stationary x tile, 512-col chunks into 2048-col psum windows)
  Each psum window is routed WHOLESALE to one consumer engine:
    A-route (ScalarE): exp with fused accum_out -> per-class piece sums
            directly (bias_b = -beta*||x_b||^2 fused; junk bf16 main out).
    C-route (VectorE): Schraudolph exp (bf16 bits = uint16(A*z + B'), 1
            tensor_scalar from psum) then one tensor_tensor_reduce per
            class piece: accum = sum(lo_half + hi_half)  (fold+reduce fused).
  Piece sums -> per-class sums (tiny DVE reduces, overlapped), logits =
  Ln(sums + eps) on ScalarE, DMA out (128, 10) per core.
  ea window DMAs alternate between the SP HWDGE queue and the GpSimd SWDGE
  queue so the 65-partition loads don't serialize on one DMA queue.
"""

import os
import numpy as np
import ml_dtypes

NUM_CLASSES = 10
EPS = 1e-12
N_CORES = 8
B = 1024
D = 64
NE = 50000
BT = 128
SEG_ALIGN = 32
CHUNK = 512
WIN = 2048             # psum window (4 banks)
SCH_A = 128.0 / float(np.log(2.0))   # bf16 exponent scale
SCH_B = 127.0 * 128.0 - 6.0          # bf16 bias + rounding tweak

# measured-on-HW per-instruction cost model (ns) used to route windows
COST_ACT_SLOPE = 1.00      # ScalarE ACTIVATE ns/col
COST_ACT_FIXED = 658.0     # per ACT instr: 313 issue + 345 accum read
COST_SCH_SLOPE = 1.20      # DVE tensor_scalar from psum (1x) ns/col
COST_TTR_SLOPE = 0.315     # DVE tensor_tensor_reduce ns per input col (2x)
COST_DVE_FIXED = 190.0

LAST_EXEC_NS = None
LAST_RESULTS = None
TRACE = bool(int(os.environ.get("KERNEL_TRACE", "0")))
TRACE_DIR = os.environ.get("KERNEL_TRACE_DIR", "")
DMA_SPLIT = int(os.environ.get("KERNEL_DMA_SPLIT", "0"))
USE_TTR = int(os.environ.get("KERNEL_TTR", "1"))
USE_WARM = int(os.environ.get("KERNEL_WARM", "1"))


def _host_prep(x, exemplars, exemplar_labels, beta_raw):
    x = np.asarray(x, dtype=np.float32)
    E = np.asarray(exemplars, dtype=np.float32)
    labels = np.asarray(exemplar_labels).astype(np.int64)
    beta = float(np.logaddexp(0.0, np.float64(beta_raw.reshape(-1)[0])))

    # global class-sorted layout with 32-aligned per-class segments
    seg_idx = []
    seg_sizes = []
    for c in range(NUM_CLASSES):
        idx_c = np.nonzero(labels == c)[0]
        seg_idx.append(idx_c)
        seg_sizes.append(max(SEG_ALIGN, int(-(-len(idx_c) // SEG_ALIGN) * SEG_ALIGN)))
    seg_offs = np.concatenate([[0], np.cumsum(seg_sizes)]).astype(np.int64)
    e_pad = int(seg_offs[-1])

    e2 = (E.astype(np.float64) ** 2).sum(axis=1)
    ea = np.zeros((D + 1, e_pad), dtype=np.float32)
    ea[D, :] = -1.0e38  # padding slots contribute exp() == 0
    for c in range(NUM_CLASSES):
        idx = seg_idx[c]
        o = int(seg_offs[c])
        ea[:D, o:o + len(idx)] = (2.0 * beta) * E[idx].T
        ea[D, o:o + len(idx)] = (-beta * e2[idx]).astype(np.float32)
    ea = ea.astype(ml_dtypes.bfloat16)

    # per-core stationary x tiles and activation biases
    xa = np.ones((D + 1, B), dtype=np.float32)
    xa[:D, :] = x.T
    xa = xa.astype(ml_dtypes.bfloat16)
    x2 = (x.astype(np.float64) ** 2).sum(axis=1)
    bias = (-beta * x2).astype(np.float32)

    hdr_cores = [
        np.ascontiguousarray(
            np.concatenate([xa[:, i * BT:(i + 1) * BT], ea[:, :CHUNK]], axis=1)
        )
        for i in range(N_CORES)
    ]
    bias_cores = [
        np.ascontiguousarray(bias[i * BT:(i + 1) * BT].reshape(BT, 1))
        for i in range(N_CORES)
    ]
    # per-partition Schraudolph affine: n = SCH_A*psum + (SCH_A*bias + SCH_B)
    schb_cores = [
        np.ascontiguousarray(
            (SCH_A * bias[i * BT:(i + 1) * BT].astype(np.float64) + SCH_B)
            .astype(np.float32)
            .reshape(BT, 1)
        )
        for i in range(N_CORES)
    ]
    return ea, hdr_cores, bias_cores, schb_cores, seg_offs, seg_sizes, e_pad


def _plan(seg_offs, e_pad):
    """Window list, per-window class pieces, and A/C engine routing."""
    wins = []
    o = 0
    for w0 in (CHUNK, 2 * CHUNK):
        if o < e_pad:
            wl = min(w0, e_pad - o)
            wins.append((o, wl))
            o += wl
    while o < e_pad:
        wins.append((o, min(WIN, e_pad - o)))
        o += wins[-1][1]

    win_pieces = []
    for (wo, wl) in wins:
        pieces = []
        for c in range(NUM_CLASSES):
            lo = max(int(seg_offs[c]), wo)
            hi = min(int(seg_offs[c + 1]), wo + wl)
            if lo < hi:
                pieces.append((c, lo, hi - lo))
        win_pieces.append(pieces)

    # greedy routing: send each window to whichever engine minimizes the
    # resulting makespan, given measured per-op costs.
    scal = 0.0
    dve = 0.0
    routes = []
    for wi, (wo, wl) in enumerate(wins):
        P = len(win_pieces[wi])
        cost_a = wl * COST_ACT_SLOPE + P * COST_ACT_FIXED
        cost_c = (wl * COST_SCH_SLOPE + COST_DVE_FIXED
                  + wl * COST_TTR_SLOPE + P * COST_DVE_FIXED)
        if max(scal + cost_a, dve) <= max(scal, dve + cost_c):
            routes.append("A")
            scal += cost_a
        else:
            routes.append("C")
            dve += cost_c
    return wins, win_pieces, routes


def _build_program(seg_offs, seg_sizes, e_pad):
    from contextlib import ExitStack
    import concourse.bass as bass
    import concourse.tile as tile
    from concourse import bacc, mybir
    import bass_rust

    f32 = mybir.dt.float32
    bf16 = mybir.dt.bfloat16

    class _Bacc(bacc.Bacc):
        # Force Exp and Ln onto the one table set that holds both, so the
        # kernel pays a single ACT_TABLE_LOAD instead of an exp-set load at
        # the start plus an ln-set load on the critical tail. Table ids are
        # positional, so positions are kept and only the choosable functions
        # are masked.
        def insert_act_table_loads(self):
            from concourse.hw_specs import get_activation_tables

            has_activation = any(
                isinstance(i, mybir.InstActivation)
                for b in self.main_func.blocks
                for i in b.instructions
            )
            if not has_activation:
                return
            E = mybir.ActivationFunctionType.Exp
            L = mybir.ActivationFunctionType.Ln
            tables = []
            for name, fns in get_activation_tables(self.m.arch).items():
                if name != "natural_log_exp_and_others":
                    fns = fns - {E, L}
                tables.append((name, fns))
            bass_rust.insert_act_table_loads(self, tables)

    nc = _Bacc(
        "TRN2",
        target_bir_lowering=False,
        debug=False,
        enable_asserts=False,
        num_devices=N_CORES,
    )

    ea_d = nc.dram_tensor("ea", [D + 1, e_pad], bf16, kind="ExternalInput").ap()
    hdr_d = nc.dram_tensor("hdr", [D + 1, BT + CHUNK], bf16, kind="ExternalInput").ap()
    bias_d = nc.dram_tensor("biasx", [BT, 1], f32, kind="ExternalInput").ap()
    schb_d = nc.dram_tensor("schb", [BT, 1], f32, kind="ExternalInput").ap()
    out_d = nc.dram_tensor("logits", [BT, NUM_CLASSES], f32, kind="ExternalOutput").ap()

    wins, win_pieces, routes = _plan(seg_offs, e_pad)

    # piece bookkeeping: column in the piece-sum tile, contiguous per class
    piece_col = {}
    n_pieces_per_class = [0] * NUM_CLASSES
    pcol = 0
    for pieces in win_pieces:
        for (c, lo, ln) in pieces:
            piece_col[(c, lo)] = pcol
            n_pieces_per_class[c] += 1
            pcol += 1
    n_pieces = pcol
    class_piece_range = []
    acc = 0
    for c in range(NUM_CLASSES):
        class_piece_range.append((acc, acc + n_pieces_per_class[c]))
        acc += n_pieces_per_class[c]

    with tile.TileContext(nc) as tc, ExitStack() as ctx:
        const_pool = ctx.enter_context(tc.tile_pool(name="const", bufs=1))
        psum_pool = ctx.enter_context(tc.tile_pool(name="psum", bufs=2, space="PSUM"))
        work_pool = ctx.enter_context(tc.tile_pool(name="work", bufs=1))

        hdr_t = const_pool.tile([D + 1, BT + CHUNK], bf16, name="hdr_t")
        nc.sync.dma_start(out=hdr_t[:], in_=hdr_d[:])
        xa_t = hdr_t[:, 0:BT]
        bias_t = const_pool.tile([BT, 1], f32, name="bias_t")
        nc.sync.dma_start(out=bias_t[:], in_=bias_d[:])
        schb_t = const_pool.tile([BT, 1], f32, name="schb_t")
        nc.sync.dma_start(out=schb_t[:], in_=schb_d[:])
        eps_t = const_pool.tile([BT, 1], f32, name="eps_t")
        nc.vector.memset(eps_t[:], float(EPS))
        warm_t = const_pool.tile([BT, 4], f32, name="warm_t")
        nc.vector.memset(warm_t[:], 0.0)

        # ea lives in a few group tiles (fewer tiles -> fewer release sems in
        # the kernel tail); each group is DMA'd in window-sized chunks,
        # alternating between the SP HWDGE queue and the GpSimd SWDGE queue.
        EA_GROUP = 8
        ea_w = [None] * len(wins)
        ea_w[0] = hdr_t[:, BT:BT + wins[0][1]]
        gi = 1
        dma_k = 0
        while gi < len(wins):
            g = wins[gi:gi + EA_GROUP]
            g_off = g[0][0]
            g_len = sum(wl for (_, wl) in g)
            t_ = const_pool.tile(
                [D + 1, g_len], bf16, name=f"ea_g{g_off}", tag=f"ea_g{g_off}"
            )
            for (wo, wl) in g:
                eng = nc.gpsimd if (DMA_SPLIT and dma_k % 2 == 1) else nc.sync
                eng.dma_start(
                    out=t_[:, wo - g_off:wo - g_off + wl], in_=ea_d[:, wo:wo + wl]
                )
                dma_k += 1
            for k, (wo, wl) in enumerate(g):
                ea_w[gi + k] = t_[:, wo - g_off:wo - g_off + wl]
            gi += len(g)

        sims = work_pool.tile([BT, WIN], bf16, name="sims")
        junk = work_pool.tile([BT, WIN], bf16, name="junk")
        jd = work_pool.tile([BT, WIN // 2], bf16, name="jd_t")
        pieces_t = work_pool.tile([BT, max(n_pieces, 1)], f32, name="pieces_t")
        cls = work_pool.tile([BT, NUM_CLASSES], f32, name="clst")
        junkf = work_pool.tile([BT, max(n_pieces, 1)], f32, name="junkf")

        # pull the exp/ln ACT table load off the critical path: a tiny dummy
        # activation issues first on ScalarE, overlapping the input DMAs.
        if USE_WARM:
            nc.scalar.activation(
                junk[:, 0:4], warm_t[:], mybir.ActivationFunctionType.Exp,
                bias=bias_t[:, 0:1], scale=1.0,
            )

        combine_done = [False] * NUM_CLASSES
        last_win_of_class = [0] * NUM_CLASSES
        for wi, pieces in enumerate(win_pieces):
            for (c, lo, ln) in pieces:
                last_win_of_class[c] = wi

        def combine(c):
            plo, phi = class_piece_range[c]
            if phi - plo > 1:
                nc.vector.tensor_scalar(
                    junkf[:, plo:phi],
                    pieces_t[:, plo:phi],
                    1.0,
                    None,
                    mybir.AluOpType.mult,
                    mybir.AluOpType.add,
                    accum_out=cls[:, c:c + 1],
                )
            else:
                nc.vector.tensor_copy(cls[:, c:c + 1], pieces_t[:, plo:plo + 1])

        for wi, (wo, wl) in enumerate(wins):
            ps = psum_pool.tile([BT, WIN], f32, tag="ps")
            co = 0
            while co < wl:
                cl = min(CHUNK, wl - co)
                nc.tensor.matmul(
                    ps[:, co:co + cl],
                    lhsT=xa_t,
                    rhs=ea_w[wi][:, co:co + cl],
                    start=True,
                    stop=True,
                )
                co += cl
            if routes[wi] == "A":
                # ScalarE: exp with fused accumulate -> piece sums directly
                for (c, lo, ln) in win_pieces[wi]:
                    pc = piece_col[(c, lo)]
                    nc.scalar.activation(
                        junk[:, :ln],
                        ps[:, lo - wo:lo - wo + ln],
                        mybir.ActivationFunctionType.Exp,
                        bias=bias_t[:, 0:1],
                        scale=1.0,
                        accum_out=pieces_t[:, pc:pc + 1],
                    )
            else:
                # VectorE: Schraudolph bf16-bits exp of the whole window,
                # then one fused fold+reduce per class piece.
                # bf16 bits = uint16(SCH_A*z + SCH_B'); uint16 saturation at
                # 0 is the exp() underflow clamp.
                nc.vector.tensor_scalar(
                    sims[:, :wl].bitcast(mybir.dt.uint16),
                    ps[:, :wl],
                    float(SCH_A),
                    schb_t[:, 0:1],
                    mybir.AluOpType.mult,
                    mybir.AluOpType.add,
                )
                for (c, lo, ln) in win_pieces[wi]:
                    pc = piece_col[(c, lo)]
                    so = lo - wo
                    h = ln // 2
                    if USE_TTR:
                        nc.vector.tensor_tensor_reduce(
                            jd[:, :h],
                            sims[:, so:so + h],
                            sims[:, so + h:so + ln],
                            1.0,
                            0.0,
                            mybir.AluOpType.add,
                            mybir.AluOpType.add,
                            accum_out=pieces_t[:, pc:pc + 1],
                        )
                    else:
                        nc.vector.tensor_add(
                            jd[:, :h], sims[:, so:so + h], sims[:, so + h:so + ln]
                        )
                        nc.vector.tensor_scalar(
                            junk[:, :h],
                            jd[:, :h],
                            1.0,
                            None,
                            mybir.AluOpType.mult,
                            mybir.AluOpType.add,
                            accum_out=pieces_t[:, pc:pc + 1],
                        )
            # fire class combines as soon as a class's last piece is done,
            # so only the final class's combine sits in the tail.
            for (c, lo, ln) in win_pieces[wi]:
                if last_win_of_class[c] == wi and not combine_done[c]:
                    combine(c)
                    combine_done[c] = True

        logit = work_pool.tile([BT, NUM_CLASSES], f32, name="logit")
        nc.scalar.activation(
            logit[:],
            cls[:],
            mybir.ActivationFunctionType.Ln,
            bias=eps_t[:, 0:1],
            scale=1.0,
        )
        nc.sync.dma_start(out=out_d, in_=logit[:])

    nc.compile()
    return nc


_PROGRAM_CACHE = {}


def kernel(x, exemplars, exemplar_labels, beta_raw):
    global LAST_EXEC_NS, LAST_RESULTS
    from concourse.bass_utils import run_bass_kernel_spmd

    ea, hdr_cores, bias_cores, schb_cores, seg_offs, seg_sizes, e_pad = _host_prep(
        x, exemplars, exemplar_labels, beta_raw
    )
    key = (tuple(seg_sizes), e_pad)
    nc = _PROGRAM_CACHE.get(key)
    if nc is None:
        nc = _build_program(seg_offs, seg_sizes, e_pad)
        _PROGRAM_CACHE[key] = nc

    in_maps = [
        {"ea": ea, "hdr": hdr_cores[i], "biasx": bias_cores[i], "schb": schb_cores[i]}
        for i in range(N_CORES)
    ]
    kwargs = {}
    if TRACE:
        kwargs["trace"] = True
        if TRACE_DIR:
            os.makedirs(TRACE_DIR, exist_ok=True)
            kwargs["tmpdir"] = TRACE_DIR
    ret = run_bass_kernel_spmd(nc, in_maps, list(range(N_CORES)), **kwargs)
    LAST_EXEC_NS = ret.exec_time_ns
    LAST_RESULTS = ret
    out = np.concatenate(
        [np.asarray(ret.results[i]["logits"], dtype=np.float32) for i in range(N_CORES)],
        axis=0,
    )
    return np.ascontiguousarray(out)
